# revision 11
# baseline (speedup 1.0000x reference)
"""AgentCollisionLoss Trainium2 kernel — PE quadratic-form formulation.

Sharding: 8 cores = B(4) x t-half(2). Core c: b = c//2, t in [40*(c%2), +40).

d2[(j),(i,k),t,l] = sq_j(l) + sq_i(k) - 2(wx_j wx_i + wy_j wy_i), one bf16
matmul per (slab-pair, l): stationary [28, 128] block-diag 2 slabs x 14
rows (strided AP picks disk-l columns from the (a,k) layout); moving
[28, 320] cols (i,k). Coords 2-way bf16 split (residual enters d2 as
2*dx*eps), sq computed FROM truncated coords, split 3-way; all bf16
products exact in fp32 PSUM. PE is throttled to 1.2 GHz on this part:
~0.83 ns/col + ~200 ns/matmul.

Operands live in two half-tiles (pairs 0-9 / 10-19) so matmuls start
after half the assembly DMAs; ones/zero DMAs are hoisted to the front
of both HWDGE queues. Drain: ACT copies PSUM->flat bf16 D, DVE XY
tensor_reduce min over (l,k). Finish interleaved in 3 chunks: clamp,
sqrt (ACT), t2 = dist*WI (host-folded W*invpd), u = W - t2,
relu+accum_out. Host sums 8 x [128, 3].
"""

import numpy as np
import ml_dtypes

import concourse.bass as bass
import concourse.bacc as bacc
import concourse.tile as tile
import concourse.mybir as mybir
from concourse import bass_utils

B, N, T, D = 4, 64, 80, 6
K = 5
NCORES = 8
BUFFER_DIST = 0.2
DECAY_RATE = 0.9
TL = T // 2          # 40 slabs per core
NPAIR = TL // 2      # 20 slab-pairs
HP = NPAIR // 2      # pairs per half-tile (10)
AK = N * K           # 320 (a,k) columns
FD = mybir.dt.float32
BF = mybir.dt.bfloat16
AF = mybir.ActivationFunctionType
AL = mybir.AluOpType
NR = 14              # contraction rows per slab
HW_ = HP * 640       # half SL width

_CACHE = {}
_LAST_INMAPS = None


def _build():
    nc = bacc.Bacc("TRN2", target_bir_lowering=False, debug=False,
                   num_devices=NCORES)

    yt_in = nc.dram_tensor("yt_in", [TL, 3 * N], FD, kind="ExternalInput").ap()
    ck_in = nc.dram_tensor("ck_in", [AK], FD, kind="ExternalInput").ap()
    wm_in = nc.dram_tensor("wm_in", [128, NPAIR * N], FD,
                           kind="ExternalInput").ap()
    wi_in = nc.dram_tensor("wi_in", [128, NPAIR * N], FD,
                           kind="ExternalInput").ap()
    z_in = nc.dram_tensor("z_in", [HW_], BF, kind="ExternalInput").ap()
    part_out = nc.dram_tensor("part_out", [128, 3], FD,
                              kind="ExternalOutput").ap()

    with tile.TileContext(nc) as tc:
        with (
            tc.tile_pool(name="prep", bufs=1) as prep,
            tc.tile_pool(name="ops", bufs=1) as ops,
            tc.tile_pool(name="fin", bufs=1) as fin,
            tc.tile_pool(name="dtile", bufs=3) as dtile,
            tc.tile_pool(name="mtmp", bufs=3) as mtmp,
            tc.tile_pool(name="p3", bufs=2, space="PSUM") as p3pool,
            tc.tile_pool(name="p2", bufs=1, space="PSUM") as p2pool,
        ):
            # SL halves: cols (p%10)*640 + h*320 + (a*5+k); V: (p%10)*320+ak
            SL0 = ops.tile([2 * NR, HW_], BF, tag="SL0")
            SL1 = ops.tile([2 * NR, HW_], BF, tag="SL1")
            V0 = ops.tile([2 * NR, HP * AK], BF, tag="V0")
            V1 = ops.tile([2 * NR, HP * AK], BF, tag="V1")
            SLh = [SL0, SL1]
            Vh = [V0, V1]

            onesA = prep.tile([TL, AK], BF)
            nc.vector.memset(onesA, 1.0)

            # zero-fill SL halves + ones rows first (queues busy from t=0)
            for i in range(2):
                nc.scalar.dma_start(
                    out=bass.AP(tensor=SLh[i].tensor, offset=SLh[i].offset,
                                ap=[SLh[i].ap[0], [1, HW_]]),
                    in_=bass.AP(tensor=z_in.tensor, offset=0,
                                ap=[[0, 2 * NR], [1, HW_]]))

            def s_dma(i, h, r, src):
                srcs = src[h * NPAIR + i * HP:h * NPAIR + (i + 1) * HP, :]
                row = SLh[i][h * NR + r:h * NR + r + 1, :]
                nc.scalar.dma_start(
                    out=bass.AP(tensor=SLh[i].tensor,
                                offset=row.offset + h * AK,
                                ap=[row.ap[0], [640, HP], [1, AK]]),
                    in_=bass.AP(tensor=srcs.tensor, offset=srcs.offset,
                                ap=[srcs.ap[0], [1, AK]]))

            def v_dma(i, h, r, src):
                srcs = src[h * NPAIR + i * HP:h * NPAIR + (i + 1) * HP, :]
                vrow = Vh[i][h * NR + r:h * NR + r + 1, :]
                nc.sync.dma_start(
                    out=bass.AP(tensor=Vh[i].tensor, offset=vrow.offset,
                                ap=[vrow.ap[0], [AK, HP], [1, AK]]),
                    in_=bass.AP(tensor=srcs.tensor, offset=srcs.offset,
                                ap=[srcs.ap[0], [1, AK]]))

            for i in range(2):
                for h in range(2):
                    for r in (3, 4, 5):
                        s_dma(i, h, r, onesA)
                    for r in (0, 1, 2):
                        v_dma(i, h, r, onesA)

            # ---------- load ----------
            YT = prep.tile([TL, 3 * N], FD)
            nc.sync.dma_start(out=YT, in_=yt_in)
            CK = prep.tile([TL, AK], FD)
            nc.sync.dma_start(
                out=CK,
                in_=bass.AP(tensor=ck_in.tensor, offset=0,
                            ap=[[0, TL], [1, AK]]))
            WM = fin.tile([128, NPAIR * N], FD)
            nc.sync.dma_start(out=WM, in_=wm_in)
            WI = fin.tile([128, NPAIR * N], FD)
            nc.sync.dma_start(out=WI, in_=wi_in)

            def colblk(t, c0, n):
                return bass.AP(tensor=t.tensor, offset=t.offset + c0,
                               ap=[t.ap[0], [1, n]])

            def bcast_ak(t, c0):     # [TL, 64] col-block -> (a,k) bcast view
                return bass.AP(tensor=t.tensor, offset=t.offset + c0,
                               ap=[t.ap[0], [1, N], [0, K]])

            pi2 = prep.tile([TL, 1], FD)
            nc.vector.memset(pi2, float(np.pi / 2))
            zb = prep.tile([TL, 1], FD)
            nc.vector.memset(zb, 0.0)

            cosT = prep.tile([TL, N], FD)
            sinT = prep.tile([TL, N], FD)
            yaw_ap = colblk(YT, 2 * N, N)
            nc.scalar.activation(out=cosT, in_=yaw_ap, func=AF.Sin, bias=pi2)
            nc.scalar.activation(out=sinT, in_=yaw_ap, func=AF.Sin, bias=zb)

            # ---------- world disk coords [TL, (a,k)] ----------
            wx = prep.tile([TL, AK], FD)
            wy = prep.tile([TL, AK], FD)
            tmp = prep.tile([TL, AK], FD)
            tmq = prep.tile([TL, AK], FD)
            nc.vector.tensor_tensor(out=tmp, in0=CK, in1=bcast_ak(cosT, 0),
                                    op=AL.mult)
            nc.vector.tensor_tensor(out=wx, in0=tmp, in1=bcast_ak(YT, 0),
                                    op=AL.add)
            nc.vector.tensor_tensor(out=tmq, in0=CK, in1=bcast_ak(sinT, 0),
                                    op=AL.mult)
            nc.vector.tensor_tensor(out=wy, in0=bcast_ak(YT, N), in1=tmq,
                                    op=AL.subtract)

            # ---------- bf16 2-way coord split ----------
            def split2(src, name):
                a = prep.tile([TL, AK], BF, tag=name + "a")
                nc.vector.tensor_copy(a, src)
                r = prep.tile([TL, AK], FD, tag=name + "r")
                nc.gpsimd.tensor_tensor(out=r, in0=src, in1=a,
                                        op=AL.subtract)
                b = prep.tile([TL, AK], BF, tag=name + "b")
                nc.vector.tensor_copy(b, r)
                trunc = prep.tile([TL, AK], FD, tag=name + "t")
                nc.gpsimd.tensor_tensor(out=trunc, in0=src, in1=r,
                                        op=AL.subtract)
                nc.vector.tensor_tensor(out=trunc, in0=trunc, in1=b,
                                        op=AL.add)
                return a, b, trunc

            xa, xb, xtr = split2(wx, "x")
            for i in range(2):
                for h in range(2):
                    s_dma(i, h, 6, xa)
                    s_dma(i, h, 7, xa)
                    s_dma(i, h, 8, xb)
                    s_dma(i, h, 9, xb)
            ya, yb, ytr = split2(wy, "y")
            for i in range(2):
                for h in range(2):
                    s_dma(i, h, 10, ya)
                    s_dma(i, h, 11, ya)
                    s_dma(i, h, 12, yb)
                    s_dma(i, h, 13, yb)

            def scale_m2(src, name):
                d = prep.tile([TL, AK], BF, tag=name)
                nc.vector.tensor_scalar(out=d, in0=src, scalar1=-2.0,
                                        scalar2=0.0, op0=AL.mult, op1=AL.add)
                return d

            m2xa = scale_m2(xa, "m2xa")
            m2xb = scale_m2(xb, "m2xb")
            for i in range(2):
                for h in range(2):
                    v_dma(i, h, 6, m2xa)
                    v_dma(i, h, 8, m2xa)
                    v_dma(i, h, 7, m2xb)
                    v_dma(i, h, 9, m2xb)
            m2ya = scale_m2(ya, "m2ya")
            m2yb = scale_m2(yb, "m2yb")
            for i in range(2):
                for h in range(2):
                    v_dma(i, h, 10, m2ya)
                    v_dma(i, h, 12, m2ya)
                    v_dma(i, h, 11, m2yb)
                    v_dma(i, h, 13, m2yb)

            # sq from truncated coords, 3-way bf16 split
            s1 = prep.tile([TL, AK], FD)
            s2 = prep.tile([TL, AK], FD)
            nc.scalar.activation(out=s1, in_=xtr, func=AF.Square, bias=zb)
            nc.scalar.activation(out=s2, in_=ytr, func=AF.Square, bias=zb)
            sq = prep.tile([TL, AK], FD)
            nc.gpsimd.tensor_tensor(out=sq, in0=s1, in1=s2, op=AL.add)

            sa = prep.tile([TL, AK], BF)
            nc.vector.tensor_copy(sa, sq)
            r2 = prep.tile([TL, AK], FD)
            nc.gpsimd.tensor_tensor(out=r2, in0=sq, in1=sa, op=AL.subtract)
            sb = prep.tile([TL, AK], BF)
            nc.vector.tensor_copy(sb, r2)
            r3 = prep.tile([TL, AK], FD)
            nc.gpsimd.tensor_tensor(out=r3, in0=r2, in1=sb, op=AL.subtract)
            sc = prep.tile([TL, AK], BF)
            nc.vector.tensor_copy(sc, r3)
            for i in range(2):
                for h in range(2):
                    s_dma(i, h, 0, sa)
                    v_dma(i, h, 3, sa)
                    s_dma(i, h, 1, sb)
                    v_dma(i, h, 4, sb)
                    s_dma(i, h, 2, sc)
                    v_dma(i, h, 5, sc)

            # ---------- main loop + interleaved finish ----------
            dmin2 = fin.tile([128, NPAIR * N], BF)
            dist = fin.tile([128, NPAIR * N], FD)
            part = fin.tile([128, 3], FD)
            zb128 = fin.tile([128, 1], FD)
            nc.vector.memset(zb128, 0.0)

            def finish_chunk(ci, p0, np_):
                c0 = p0 * N
                w = np_ * N
                dsl = bass.AP(tensor=dmin2.tensor, offset=dmin2.offset + c0,
                              ap=[dmin2.ap[0], [1, w]])
                nc.vector.tensor_scalar(out=dsl, in0=dsl, scalar1=0.0,
                                        scalar2=None, op0=AL.max)
                dstl = bass.AP(tensor=dist.tensor, offset=dist.offset + c0,
                               ap=[dist.ap[0], [1, w]])
                nc.scalar.activation(out=dstl, in_=dsl, func=AF.Sqrt,
                                     bias=zb128)
                t2 = mtmp.tile([128, w], FD, tag=f"t2{ci}")
                u = mtmp.tile([128, w], FD, tag=f"u{ci}")
                wi_ap = bass.AP(tensor=WI.tensor, offset=WI.offset + c0,
                                ap=[WI.ap[0], [1, w]])
                wm_ap = bass.AP(tensor=WM.tensor, offset=WM.offset + c0,
                                ap=[WM.ap[0], [1, w]])
                if ci == 0:
                    nc.gpsimd.tensor_tensor(out=t2, in0=dstl, in1=wi_ap,
                                            op=AL.mult)
                    nc.gpsimd.tensor_tensor(out=u, in0=wm_ap, in1=t2,
                                            op=AL.subtract)
                else:
                    nc.vector.tensor_tensor(out=t2, in0=dstl, in1=wi_ap,
                                            op=AL.mult)
                    nc.vector.tensor_tensor(out=u, in0=wm_ap, in1=t2,
                                            op=AL.subtract)
                nc.scalar.activation(out=u, in_=u, func=AF.Relu, bias=zb128,
                                     accum_out=part[:, ci:ci + 1])

            for p in range(NPAIR):
                i, pl = divmod(p, HP)
                P3 = p3pool.tile([128, 3 * 512], FD, tag="P3")
                P2 = p2pool.tile([128, 2 * 512], FD, tag="P2")
                for l in range(5):
                    dst = P3 if l < 3 else P2
                    c0 = 512 * l if l < 3 else 512 * (l - 3)
                    nc.tensor.matmul(
                        out=dst[0:128, c0:c0 + AK],
                        lhsT=bass.AP(tensor=SLh[i].tensor,
                                     offset=SLh[i].offset + 640 * pl + l,
                                     ap=[SLh[i].ap[0], [AK, 2], [K, N]]),
                        rhs=Vh[i][0:2 * NR, AK * pl:AK * (pl + 1)],
                        tile_position=(0, 0))

                dslice = bass.AP(tensor=dmin2.tensor,
                                 offset=dmin2.offset + p * N,
                                 ap=[dmin2.ap[0], [1, N]])
                # flat bf16 D: cols l*320 + i*5 + k
                Dt = dtile.tile([128, 5 * AK], BF, tag="D")
                nc.scalar.activation(
                    out=bass.AP(tensor=Dt.tensor, offset=Dt.offset,
                                ap=[Dt.ap[0], [AK, 3], [1, AK]]),
                    in_=bass.AP(tensor=P3.tensor, offset=P3.offset,
                                ap=[P3.ap[0], [512, 3], [1, AK]]),
                    func=AF.Copy)
                nc.scalar.activation(
                    out=bass.AP(tensor=Dt.tensor, offset=Dt.offset + 3 * AK,
                                ap=[Dt.ap[0], [AK, 2], [1, AK]]),
                    in_=bass.AP(tensor=P2.tensor, offset=P2.offset,
                                ap=[P2.ap[0], [512, 2], [1, AK]]),
                    func=AF.Copy)
                nc.vector.tensor_reduce(
                    out=dslice,
                    in_=bass.AP(tensor=Dt.tensor, offset=Dt.offset,
                                ap=[Dt.ap[0], [K, N], [AK, 5], [1, K]]),
                    axis=mybir.AxisListType.XY, op=AL.min)

                if p == 9:
                    finish_chunk(0, 0, 10)
                elif p == 18:
                    finish_chunk(1, 10, 9)
                elif p == 19:
                    finish_chunk(2, 19, 1)

            nc.sync.dma_start(out=part_out, in_=part)

    nc.compile()
    return nc


def kernel(Y, length, width):
    Y = np.asarray(Y, np.float32)
    length = np.asarray(length, np.float32)
    width = np.asarray(width, np.float32)

    if "nc" not in _CACHE:
        _CACHE["nc"] = _build()
    nc = _CACHE["nc"]

    f2 = (2.0 * np.arange(K, dtype=np.float32) / (K - 1) - 1.0)
    ew = DECAY_RATE ** np.arange(T, dtype=np.float32)
    ew = (ew / ew.sum()).astype(np.float64)

    # prep-row rr = h*20 + p  <->  local slab t_local = 2p + h
    rr = np.arange(TL)
    tl_of_rr = 2 * (rr % NPAIR) + rr // NPAIR

    in_maps = []
    for c in range(NCORES):
        b, th = divmod(c, 2)
        t0 = th * TL
        tglob = t0 + tl_of_rr                       # [TL] global t per row

        yt = np.empty((TL, 3 * N), np.float32)
        yt[:, 0:N] = Y[b, :, tglob, 0]              # x[t, a]
        yt[:, N:2 * N] = Y[b, :, tglob, 1]          # y
        yt[:, 2 * N:3 * N] = Y[b, :, tglob, 4]      # yaw

        rad = width[b] / 2.0
        cmax = length[b] / 2.0 - rad                # [N]
        ck = (cmax[:, None] * f2[None, :]).reshape(AK).astype(np.float32)

        pd = rad[:, None] + rad[None, :] + BUFFER_DIST   # [j, i]
        ip = np.concatenate([1.0 / pd, 1.0 / pd], axis=0)  # [128, 64]

        wm = np.zeros((128, NPAIR * N), np.float64)
        mask = (~np.eye(N, dtype=bool)).astype(np.float64)   # [j, i]
        for p in range(NPAIR):
            for h in range(2):
                t = t0 + 2 * p + h
                wm[h * N:(h + 1) * N, p * N:(p + 1) * N] = \
                    mask * (ew[t] / (B * N * T))
        wi = wm * np.tile(ip, (1, NPAIR)).astype(np.float64)

        in_maps.append({
            "yt_in": yt, "ck_in": ck,
            "wm_in": wm.astype(np.float32),
            "wi_in": wi.astype(np.float32),
            "z_in": np.zeros(HW_, ml_dtypes.bfloat16),
        })

    global _LAST_INMAPS
    _LAST_INMAPS = in_maps
    res = bass_utils.run_bass_kernel_spmd(nc, in_maps,
                                          core_ids=list(range(NCORES)))
    total = 0.0
    for c in range(NCORES):
        total += float(res.results[c]["part_out"].astype(np.float64).sum())
    return np.float32(total)


# revision 13
# speedup vs baseline: 1.2003x; 1.2003x over previous
"""AgentCollisionLoss Trainium2 kernel — PE quadratic-form formulation.

Sharding: 8 cores = B(4) x t-half(2). Core c: b = c//2, t in [40*(c%2), +40).

d2[(j),(i,k),t,l] = sq_j(l) + sq_i(k) - 2(wx_j wx_i + wy_j wy_i), one bf16
matmul per (slab-pair, l): stationary [20, 128] block-diag 2 slabs x 10
rows (strided AP picks disk-l columns from the (a,k) layout); moving
[20, 320] cols (i,k). Coords 2-way bf16 split (xa = bf16(wx),
xb = bf16(wx - xa)); sq computed FROM the truncated coords and split
2-way. Rows per slab: (sa,1),(sb,1),(1,sa),(1,sb),(xa,-2xa),(xa,-2xb),
(xb,-2xa),(ya,-2ya),(ya,-2yb),(yb,-2ya). Dropped terms (sq 3rd split
~0.1, xb*xb ~0.02) stay well inside the 2e-2 tolerance. PE is locked
at 1.2 GHz here: ~0.83 ns/col + ~200 ns/matmul -> ~40 us of matmul.

Drain: ACT copies PSUM->flat bf16 D [128,(l,i,k)], DVE XY tensor_reduce
min over (l,k). Finish interleaved in 3 chunks: clamp, sqrt (ACT),
t2 = dist*WI (host-folded W*invpd), u = W - t2, relu+accum_out.
Host sums 8 x [128, 3].
"""

import numpy as np
import ml_dtypes

import concourse.bass as bass
import concourse.bacc as bacc
import concourse.tile as tile
import concourse.mybir as mybir
from concourse import bass_utils

B, N, T, D = 4, 64, 80, 6
K = 5
NCORES = 8
BUFFER_DIST = 0.2
DECAY_RATE = 0.9
TL = T // 2          # 40 slabs per core
NPAIR = TL // 2      # 20 slab-pairs
AK = N * K           # 320 (a,k) columns
FD = mybir.dt.float32
BF = mybir.dt.bfloat16
AF = mybir.ActivationFunctionType
AL = mybir.AluOpType
NR = 10              # contraction rows per slab
SLW = NPAIR * 640    # SL cols: p*640 + h*320 + (a*5+k)

_CACHE = {}
_LAST_INMAPS = None


def _build():
    nc = bacc.Bacc("TRN2", target_bir_lowering=False, debug=False,
                   num_devices=NCORES)

    yt_in = nc.dram_tensor("yt_in", [TL, 3 * N], FD, kind="ExternalInput").ap()
    ck_in = nc.dram_tensor("ck_in", [AK], FD, kind="ExternalInput").ap()
    wm_in = nc.dram_tensor("wm_in", [128, NPAIR * N], FD,
                           kind="ExternalInput").ap()
    wi_in = nc.dram_tensor("wi_in", [128, NPAIR * N], FD,
                           kind="ExternalInput").ap()
    z_in = nc.dram_tensor("z_in", [SLW], BF, kind="ExternalInput").ap()
    part_out = nc.dram_tensor("part_out", [128, 3], FD,
                              kind="ExternalOutput").ap()

    with tile.TileContext(nc) as tc:
        with (
            tc.tile_pool(name="prep", bufs=1) as prep,
            tc.tile_pool(name="ops", bufs=1) as ops,
            tc.tile_pool(name="fin", bufs=1) as fin,
            tc.tile_pool(name="dtile", bufs=3) as dtile,
            tc.tile_pool(name="mtmp", bufs=3) as mtmp,
            tc.tile_pool(name="p3", bufs=2, space="PSUM") as p3pool,
            tc.tile_pool(name="p2", bufs=1, space="PSUM") as p2pool,
        ):
            SL = ops.tile([2 * NR, SLW], BF)
            V = ops.tile([2 * NR, NPAIR * AK], BF)

            onesA = prep.tile([TL, AK], BF)
            nc.vector.memset(onesA, 1.0)

            # early queue work: zero-fill SL (sync), ones rows
            nc.sync.dma_start(
                out=bass.AP(tensor=SL.tensor, offset=SL.offset,
                            ap=[SL.ap[0], [1, SLW]]),
                in_=bass.AP(tensor=z_in.tensor, offset=0,
                            ap=[[0, 2 * NR], [1, SLW]]))

            def s_dma(h, r, src, eng):
                srcs = src[h * NPAIR:(h + 1) * NPAIR, :]
                row = SL[h * NR + r:h * NR + r + 1, :]
                eng.dma_start(
                    out=bass.AP(tensor=SL.tensor,
                                offset=row.offset + h * AK,
                                ap=[row.ap[0], [640, NPAIR], [1, AK]]),
                    in_=bass.AP(tensor=srcs.tensor, offset=srcs.offset,
                                ap=[srcs.ap[0], [1, AK]]))

            def v_dma(h, r, src, eng):
                srcs = src[h * NPAIR:(h + 1) * NPAIR, :]
                vrow = V[h * NR + r:h * NR + r + 1, :]
                eng.dma_start(
                    out=bass.AP(tensor=V.tensor, offset=vrow.offset,
                                ap=[vrow.ap[0], [AK, NPAIR], [1, AK]]),
                    in_=bass.AP(tensor=srcs.tensor, offset=srcs.offset,
                                ap=[srcs.ap[0], [1, AK]]))

            # ones rows: S r2,r3; V r0,r1  (8 DMAs on sync, ready at t0)
            for h in range(2):
                for r in (2, 3):
                    s_dma(h, r, onesA, nc.sync)
                for r in (0, 1):
                    v_dma(h, r, onesA, nc.sync)

            # ---------- load ----------
            YT = prep.tile([TL, 3 * N], FD)
            nc.sync.dma_start(out=YT, in_=yt_in)
            CK = prep.tile([TL, AK], FD)
            nc.sync.dma_start(
                out=CK,
                in_=bass.AP(tensor=ck_in.tensor, offset=0,
                            ap=[[0, TL], [1, AK]]))
            WM = fin.tile([128, NPAIR * N], FD)
            nc.sync.dma_start(out=WM, in_=wm_in)
            WI = fin.tile([128, NPAIR * N], FD)
            nc.sync.dma_start(out=WI, in_=wi_in)

            def colblk(t, c0, n):
                return bass.AP(tensor=t.tensor, offset=t.offset + c0,
                               ap=[t.ap[0], [1, n]])

            def bcast_ak(t, c0):     # [TL, 64] col-block -> (a,k) bcast view
                return bass.AP(tensor=t.tensor, offset=t.offset + c0,
                               ap=[t.ap[0], [1, N], [0, K]])

            pi2 = prep.tile([TL, 1], FD)
            nc.vector.memset(pi2, float(np.pi / 2))
            zb = prep.tile([TL, 1], FD)
            nc.vector.memset(zb, 0.0)

            cosT = prep.tile([TL, N], FD)
            sinT = prep.tile([TL, N], FD)
            yaw_ap = colblk(YT, 2 * N, N)
            nc.scalar.activation(out=cosT, in_=yaw_ap, func=AF.Sin, bias=pi2)
            nc.scalar.activation(out=sinT, in_=yaw_ap, func=AF.Sin, bias=zb)

            # ---------- world disk coords [TL, (a,k)] ----------
            wx = prep.tile([TL, AK], FD)
            wy = prep.tile([TL, AK], FD)
            tmp = prep.tile([TL, AK], FD)
            tmq = prep.tile([TL, AK], FD)
            nc.vector.tensor_tensor(out=tmp, in0=CK, in1=bcast_ak(cosT, 0),
                                    op=AL.mult)
            nc.vector.tensor_tensor(out=wx, in0=tmp, in1=bcast_ak(YT, 0),
                                    op=AL.add)
            nc.vector.tensor_tensor(out=tmq, in0=CK, in1=bcast_ak(sinT, 0),
                                    op=AL.mult)
            nc.vector.tensor_tensor(out=wy, in0=bcast_ak(YT, N), in1=tmq,
                                    op=AL.subtract)

            # ---------- bf16 2-way coord split ----------
            def split2(src, name):
                a = prep.tile([TL, AK], BF, tag=name + "a")
                nc.vector.tensor_copy(a, src)
                r = prep.tile([TL, AK], FD, tag=name + "r")
                nc.gpsimd.tensor_tensor(out=r, in0=src, in1=a,
                                        op=AL.subtract)
                b = prep.tile([TL, AK], BF, tag=name + "b")
                nc.vector.tensor_copy(b, r)
                trunc = prep.tile([TL, AK], FD, tag=name + "t")
                nc.gpsimd.tensor_tensor(out=trunc, in0=src, in1=r,
                                        op=AL.subtract)
                nc.vector.tensor_tensor(out=trunc, in0=trunc, in1=b,
                                        op=AL.add)
                return a, b, trunc

            def scale_m2(src, name):
                d = prep.tile([TL, AK], BF, tag=name)
                nc.vector.tensor_scalar(out=d, in0=src, scalar1=-2.0,
                                        scalar2=0.0, op0=AL.mult, op1=AL.add)
                return d

            # x side: S rows r4,r5 = xa, r6 = xb; V rows r4 = -2xa,
            # r5 = -2xb, r6 = -2xa
            xa, xb, xtr = split2(wx, "x")
            for h in range(2):
                s_dma(h, 4, xa, nc.sync)
                s_dma(h, 5, xa, nc.sync)
                s_dma(h, 6, xb, nc.sync)
            m2xa = scale_m2(xa, "m2xa")
            m2xb = scale_m2(xb, "m2xb")
            for h in range(2):
                v_dma(h, 4, m2xa, nc.scalar)
                v_dma(h, 6, m2xa, nc.scalar)
                v_dma(h, 5, m2xb, nc.scalar)

            # y side: S rows r7,r8 = ya, r9 = yb; V r7 = -2ya, r8 = -2yb,
            # r9 = -2ya
            ya, yb, ytr = split2(wy, "y")
            for h in range(2):
                s_dma(h, 7, ya, nc.sync)
                s_dma(h, 8, ya, nc.sync)
                s_dma(h, 9, yb, nc.sync)
            m2ya = scale_m2(ya, "m2ya")
            m2yb = scale_m2(yb, "m2yb")
            for h in range(2):
                v_dma(h, 7, m2ya, nc.scalar)
                v_dma(h, 9, m2ya, nc.scalar)
                v_dma(h, 8, m2yb, nc.scalar)

            # sq from truncated coords, 2-way bf16 split
            s1 = prep.tile([TL, AK], FD)
            s2 = prep.tile([TL, AK], FD)
            nc.scalar.activation(out=s1, in_=xtr, func=AF.Square, bias=zb)
            nc.scalar.activation(out=s2, in_=ytr, func=AF.Square, bias=zb)
            sq = prep.tile([TL, AK], FD)
            nc.gpsimd.tensor_tensor(out=sq, in0=s1, in1=s2, op=AL.add)

            sa = prep.tile([TL, AK], BF)
            nc.vector.tensor_copy(sa, sq)
            r2 = prep.tile([TL, AK], FD)
            nc.gpsimd.tensor_tensor(out=r2, in0=sq, in1=sa, op=AL.subtract)
            sb = prep.tile([TL, AK], BF)
            nc.vector.tensor_copy(sb, r2)
            for h in range(2):
                s_dma(h, 0, sa, nc.scalar)
                v_dma(h, 2, sa, nc.sync)
                s_dma(h, 1, sb, nc.scalar)
                v_dma(h, 3, sb, nc.sync)

            # ---------- main loop + interleaved finish ----------
            dmin2 = fin.tile([128, NPAIR * N], BF)
            dist = fin.tile([128, NPAIR * N], FD)
            part = fin.tile([128, 3], FD)
            zb128 = fin.tile([128, 1], FD)
            nc.vector.memset(zb128, 0.0)

            def finish_chunk(ci, p0, np_):
                c0 = p0 * N
                w = np_ * N
                dsl = bass.AP(tensor=dmin2.tensor, offset=dmin2.offset + c0,
                              ap=[dmin2.ap[0], [1, w]])
                nc.vector.tensor_scalar(out=dsl, in0=dsl, scalar1=0.0,
                                        scalar2=None, op0=AL.max)
                dstl = bass.AP(tensor=dist.tensor, offset=dist.offset + c0,
                               ap=[dist.ap[0], [1, w]])
                nc.scalar.activation(out=dstl, in_=dsl, func=AF.Sqrt,
                                     bias=zb128)
                t2 = mtmp.tile([128, w], FD, tag=f"t2{ci}")
                u = mtmp.tile([128, w], FD, tag=f"u{ci}")
                wi_ap = bass.AP(tensor=WI.tensor, offset=WI.offset + c0,
                                ap=[WI.ap[0], [1, w]])
                wm_ap = bass.AP(tensor=WM.tensor, offset=WM.offset + c0,
                                ap=[WM.ap[0], [1, w]])
                if ci == 0:
                    nc.gpsimd.tensor_tensor(out=t2, in0=dstl, in1=wi_ap,
                                            op=AL.mult)
                    nc.gpsimd.tensor_tensor(out=u, in0=wm_ap, in1=t2,
                                            op=AL.subtract)
                else:
                    nc.vector.tensor_tensor(out=t2, in0=dstl, in1=wi_ap,
                                            op=AL.mult)
                    nc.vector.tensor_tensor(out=u, in0=wm_ap, in1=t2,
                                            op=AL.subtract)
                nc.scalar.activation(out=u, in_=u, func=AF.Relu, bias=zb128,
                                     accum_out=part[:, ci:ci + 1])

            for p in range(NPAIR):
                P3 = p3pool.tile([128, 3 * 512], FD, tag="P3")
                P2 = p2pool.tile([128, 2 * 512], FD, tag="P2")
                for l in range(5):
                    dst = P3 if l < 3 else P2
                    c0 = 512 * l if l < 3 else 512 * (l - 3)
                    nc.tensor.matmul(
                        out=dst[0:128, c0:c0 + AK],
                        lhsT=bass.AP(tensor=SL.tensor,
                                     offset=SL.offset + 640 * p + l,
                                     ap=[SL.ap[0], [AK, 2], [K, N]]),
                        rhs=V[0:2 * NR, AK * p:AK * (p + 1)],
                        tile_position=(0, 0))

                dslice = bass.AP(tensor=dmin2.tensor,
                                 offset=dmin2.offset + p * N,
                                 ap=[dmin2.ap[0], [1, N]])
                # flat bf16 D: cols l*320 + i*5 + k
                Dt = dtile.tile([128, 5 * AK], BF, tag="D")
                nc.scalar.activation(
                    out=bass.AP(tensor=Dt.tensor, offset=Dt.offset,
                                ap=[Dt.ap[0], [AK, 3], [1, AK]]),
                    in_=bass.AP(tensor=P3.tensor, offset=P3.offset,
                                ap=[P3.ap[0], [512, 3], [1, AK]]),
                    func=AF.Copy)
                nc.scalar.activation(
                    out=bass.AP(tensor=Dt.tensor, offset=Dt.offset + 3 * AK,
                                ap=[Dt.ap[0], [AK, 2], [1, AK]]),
                    in_=bass.AP(tensor=P2.tensor, offset=P2.offset,
                                ap=[P2.ap[0], [512, 2], [1, AK]]),
                    func=AF.Copy)
                nc.vector.tensor_reduce(
                    out=dslice,
                    in_=bass.AP(tensor=Dt.tensor, offset=Dt.offset,
                                ap=[Dt.ap[0], [K, N], [AK, 5], [1, K]]),
                    axis=mybir.AxisListType.XY, op=AL.min)

                if p == 9:
                    finish_chunk(0, 0, 10)
                elif p == 18:
                    finish_chunk(1, 10, 9)
                elif p == 19:
                    finish_chunk(2, 19, 1)

            nc.sync.dma_start(out=part_out, in_=part)

    nc.compile()
    return nc


def kernel(Y, length, width):
    Y = np.asarray(Y, np.float32)
    length = np.asarray(length, np.float32)
    width = np.asarray(width, np.float32)

    if "nc" not in _CACHE:
        _CACHE["nc"] = _build()
    nc = _CACHE["nc"]

    f2 = (2.0 * np.arange(K, dtype=np.float32) / (K - 1) - 1.0)
    ew = DECAY_RATE ** np.arange(T, dtype=np.float32)
    ew = (ew / ew.sum()).astype(np.float64)

    # prep-row rr = h*20 + p  <->  local slab t_local = 2p + h
    rr = np.arange(TL)
    tl_of_rr = 2 * (rr % NPAIR) + rr // NPAIR

    in_maps = []
    for c in range(NCORES):
        b, th = divmod(c, 2)
        t0 = th * TL
        tglob = t0 + tl_of_rr                       # [TL] global t per row

        yt = np.empty((TL, 3 * N), np.float32)
        yt[:, 0:N] = Y[b, :, tglob, 0]              # x[t, a]
        yt[:, N:2 * N] = Y[b, :, tglob, 1]          # y
        yt[:, 2 * N:3 * N] = Y[b, :, tglob, 4]      # yaw

        rad = width[b] / 2.0
        cmax = length[b] / 2.0 - rad                # [N]
        ck = (cmax[:, None] * f2[None, :]).reshape(AK).astype(np.float32)

        pd = rad[:, None] + rad[None, :] + BUFFER_DIST   # [j, i]
        ip = np.concatenate([1.0 / pd, 1.0 / pd], axis=0)  # [128, 64]

        wm = np.zeros((128, NPAIR * N), np.float64)
        mask = (~np.eye(N, dtype=bool)).astype(np.float64)   # [j, i]
        for p in range(NPAIR):
            for h in range(2):
                t = t0 + 2 * p + h
                wm[h * N:(h + 1) * N, p * N:(p + 1) * N] = \
                    mask * (ew[t] / (B * N * T))
        wi = wm * np.tile(ip, (1, NPAIR)).astype(np.float64)

        in_maps.append({
            "yt_in": yt, "ck_in": ck,
            "wm_in": wm.astype(np.float32),
            "wi_in": wi.astype(np.float32),
            "z_in": np.zeros(SLW, ml_dtypes.bfloat16),
        })

    global _LAST_INMAPS
    _LAST_INMAPS = in_maps
    res = bass_utils.run_bass_kernel_spmd(nc, in_maps,
                                          core_ids=list(range(NCORES)))
    total = 0.0
    for c in range(NCORES):
        total += float(res.results[c]["part_out"].astype(np.float64).sum())
    return np.float32(total)


# revision 16
# speedup vs baseline: 1.3437x; 1.1194x over previous
"""AgentCollisionLoss Trainium2 kernel — PE quadratic-form formulation.

Sharding: 8 cores = B(4) x t-half(2). Core c: b = c//2, t in [40*(c%2), +40).

d2[(j),(i,k),t,l] = sq_j(l) + sq_i(k) - 2(wx_j wx_i + wy_j wy_i), one bf16
matmul per (slab-pair, l): stationary [20, 128] block-diag 2 slabs x 10
rows (strided AP picks disk-l columns from the (a,k) layout); moving
[20, 320] cols (i,k). Coords 2-way bf16 split (xa = bf16(wx),
xb = bf16(wx - xa)); sq computed FROM the truncated coords and split
2-way. Rows per slab: (sa,1),(sb,1),(1,sa),(1,sb),(xa,-2xa),(xa,-2xb),
(xb,-2xa),(ya,-2ya),(ya,-2yb),(yb,-2ya). Dropped terms (sq 3rd split
~0.1, xb*xb ~0.02) stay well inside the 2e-2 tolerance. PE is locked
at 1.2 GHz here: ~0.83 ns/col + ~200 ns/matmul -> ~40 us of matmul.

Drain: ACT copies PSUM->flat bf16 D [128,(l,i,k)], DVE XY tensor_reduce
min over (l,k). Finish interleaved in 3 chunks: clamp, sqrt (ACT),
t2 = dist*WI (host-folded W*invpd), u = W - t2, relu+accum_out.
Host sums 8 x [128, 3].
"""

import numpy as np
import ml_dtypes

import concourse.bass as bass
import concourse.bacc as bacc
import concourse.tile as tile
import concourse.mybir as mybir
from concourse import bass_utils

B, N, T, D = 4, 64, 80, 6
K = 5
NCORES = 8
BUFFER_DIST = 0.2
DECAY_RATE = 0.9
TL = T // 2          # 40 slabs per core
NPAIR = TL // 2      # 20 slab-pairs
AK = N * K           # 320 (a,k) columns
FD = mybir.dt.float32
BF = mybir.dt.bfloat16
AF = mybir.ActivationFunctionType
AL = mybir.AluOpType
NR = 10              # contraction rows per slab
SLW = NPAIR * 640    # SL cols: p*640 + h*320 + (a*5+k)

_CACHE = {}
_LAST_INMAPS = None


def _build():
    nc = bacc.Bacc("TRN2", target_bir_lowering=False, debug=False,
                   num_devices=NCORES)

    yt_in = nc.dram_tensor("yt_in", [TL, 3 * N], FD, kind="ExternalInput").ap()
    ck_in = nc.dram_tensor("ck_in", [AK], FD, kind="ExternalInput").ap()
    wm_in = nc.dram_tensor("wm_in", [128, NPAIR * N], FD,
                           kind="ExternalInput").ap()
    wi_in = nc.dram_tensor("wi_in", [128, NPAIR * N], FD,
                           kind="ExternalInput").ap()
    zi_in = nc.dram_tensor("zi_in", [2 * NR, SLW], BF,
                           kind="ExternalInput").ap()
    on_in = nc.dram_tensor("on_in", [NPAIR * AK], BF,
                           kind="ExternalInput").ap()
    part_out = nc.dram_tensor("part_out", [128, 3], FD,
                              kind="ExternalOutput").ap()

    with tile.TileContext(nc) as tc:
        with (
            tc.tile_pool(name="prep", bufs=1) as prep,
            tc.tile_pool(name="ops", bufs=1) as ops,
            tc.tile_pool(name="fin", bufs=1) as fin,
            tc.tile_pool(name="dtile", bufs=3) as dtile,
            tc.tile_pool(name="mtmp", bufs=3) as mtmp,
            tc.tile_pool(name="p3", bufs=2, space="PSUM") as p3pool,
            tc.tile_pool(name="p2", bufs=1, space="PSUM") as p2pool,
        ):
            SL = ops.tile([2 * NR, SLW], BF)
            V = ops.tile([2 * NR, NPAIR * AK], BF)

            def s_dma(h, r, src, eng):
                srcs = src[h * NPAIR:(h + 1) * NPAIR, :]
                row = SL[h * NR + r:h * NR + r + 1, :]
                eng.dma_start(
                    out=bass.AP(tensor=SL.tensor,
                                offset=row.offset + h * AK,
                                ap=[row.ap[0], [640, NPAIR], [1, AK]]),
                    in_=bass.AP(tensor=srcs.tensor, offset=srcs.offset,
                                ap=[srcs.ap[0], [1, AK]]))

            def v_dma(h, r, src, eng):
                srcs = src[h * NPAIR:(h + 1) * NPAIR, :]
                vrow = V[h * NR + r:h * NR + r + 1, :]
                eng.dma_start(
                    out=bass.AP(tensor=V.tensor, offset=vrow.offset,
                                ap=[vrow.ap[0], [AK, NPAIR], [1, AK]]),
                    in_=bass.AP(tensor=srcs.tensor, offset=srcs.offset,
                                ap=[srcs.ap[0], [1, AK]]))

            # ---------- loads first (gate the compute chain) ----------
            YT = prep.tile([TL, 3 * N], FD)
            nc.sync.dma_start(out=YT, in_=yt_in)
            CK = prep.tile([TL, AK], FD)
            nc.sync.dma_start(
                out=CK,
                in_=bass.AP(tensor=ck_in.tensor, offset=0,
                            ap=[[0, TL], [1, AK]]))
            # SL constants (zeros + ones rows) in one image DMA
            nc.sync.dma_start(
                out=SL,
                in_=bass.AP(tensor=zi_in.tensor, offset=0,
                            ap=[[SLW, 2 * NR], [1, SLW]]))
            # V ones rows r0,r1 per h
            for h in range(2):
                vr = V[h * NR:h * NR + 2, :]
                nc.scalar.dma_start(
                    out=vr,
                    in_=bass.AP(tensor=on_in.tensor, offset=0,
                                ap=[[0, 2], [1, NPAIR * AK]]))
            WM = fin.tile([128, NPAIR * N], FD)
            nc.sync.dma_start(out=WM, in_=wm_in)
            WI = fin.tile([128, NPAIR * N], FD)
            nc.sync.dma_start(out=WI, in_=wi_in)

            def colblk(t, c0, n):
                return bass.AP(tensor=t.tensor, offset=t.offset + c0,
                               ap=[t.ap[0], [1, n]])

            def bcast_ak(t, c0):     # [TL, 64] col-block -> (a,k) bcast view
                return bass.AP(tensor=t.tensor, offset=t.offset + c0,
                               ap=[t.ap[0], [1, N], [0, K]])

            pi2 = prep.tile([TL, 1], FD)
            nc.vector.memset(pi2, float(np.pi / 2))
            zb = prep.tile([TL, 1], FD)
            nc.vector.memset(zb, 0.0)

            cosT = prep.tile([TL, N], FD)
            sinT = prep.tile([TL, N], FD)
            yaw_ap = colblk(YT, 2 * N, N)
            nc.scalar.activation(out=cosT, in_=yaw_ap, func=AF.Sin, bias=pi2)
            nc.scalar.activation(out=sinT, in_=yaw_ap, func=AF.Sin, bias=zb)

            # ---------- world disk coords [TL, (a,k)] ----------
            wx = prep.tile([TL, AK], FD)
            wy = prep.tile([TL, AK], FD)
            tmp = prep.tile([TL, AK], FD)
            tmq = prep.tile([TL, AK], FD)
            nc.vector.tensor_tensor(out=tmp, in0=CK, in1=bcast_ak(cosT, 0),
                                    op=AL.mult)
            nc.vector.tensor_tensor(out=wx, in0=tmp, in1=bcast_ak(YT, 0),
                                    op=AL.add)
            nc.vector.tensor_tensor(out=tmq, in0=CK, in1=bcast_ak(sinT, 0),
                                    op=AL.mult)
            nc.vector.tensor_tensor(out=wy, in0=bcast_ak(YT, N), in1=tmq,
                                    op=AL.subtract)

            # ---------- squares first (longest chain), then splits ----------
            s1 = prep.tile([TL, AK], FD)
            s2 = prep.tile([TL, AK], FD)
            nc.scalar.activation(out=s1, in_=wx, func=AF.Square, bias=zb)
            nc.scalar.activation(out=s2, in_=wy, func=AF.Square, bias=zb)
            sq = prep.tile([TL, AK], FD)
            nc.gpsimd.tensor_tensor(out=sq, in0=s1, in1=s2, op=AL.add)
            sa = prep.tile([TL, AK], BF)
            nc.vector.tensor_copy(sa, sq)
            r2 = prep.tile([TL, AK], FD)
            nc.gpsimd.tensor_tensor(out=r2, in0=sq, in1=sa, op=AL.subtract)
            sb = prep.tile([TL, AK], BF)
            nc.vector.tensor_copy(sb, r2)
            for h in range(2):
                s_dma(h, 0, sa, nc.scalar)
                v_dma(h, 2, sa, nc.sync)
                s_dma(h, 1, sb, nc.scalar)
                v_dma(h, 3, sb, nc.sync)

            def split2(src_, name):
                a = prep.tile([TL, AK], BF, tag=name + "a")
                nc.vector.tensor_copy(a, src_)
                r = prep.tile([TL, AK], FD, tag=name + "r")
                nc.gpsimd.tensor_tensor(out=r, in0=src_, in1=a,
                                        op=AL.subtract)
                b = prep.tile([TL, AK], BF, tag=name + "b")
                nc.vector.tensor_copy(b, r)
                return a, b

            def scale_m2(src_, name):
                d = prep.tile([TL, AK], BF, tag=name)
                nc.vector.tensor_scalar(out=d, in0=src_, scalar1=-2.0,
                                        scalar2=0.0, op0=AL.mult, op1=AL.add)
                return d

            # x side: S rows r4,r5 = xa, r6 = xb; V r4 = -2xa, r5 = -2xb,
            # r6 = -2xa
            xa, xb = split2(wx, "x")
            for h in range(2):
                s_dma(h, 4, xa, nc.sync)
                s_dma(h, 5, xa, nc.sync)
                s_dma(h, 6, xb, nc.sync)
            m2xa = scale_m2(xa, "m2xa")
            m2xb = scale_m2(xb, "m2xb")
            for h in range(2):
                v_dma(h, 4, m2xa, nc.scalar)
                v_dma(h, 6, m2xa, nc.scalar)
                v_dma(h, 5, m2xb, nc.scalar)

            # y side: S rows r7,r8 = ya, r9 = yb; V r7 = -2ya, r8 = -2yb,
            # r9 = -2ya
            ya, yb = split2(wy, "y")
            for h in range(2):
                s_dma(h, 7, ya, nc.sync)
                s_dma(h, 8, ya, nc.sync)
                s_dma(h, 9, yb, nc.sync)
            m2ya = scale_m2(ya, "m2ya")
            m2yb = scale_m2(yb, "m2yb")
            for h in range(2):
                v_dma(h, 7, m2ya, nc.scalar)
                v_dma(h, 9, m2ya, nc.scalar)
                v_dma(h, 8, m2yb, nc.scalar)

            # ---------- main loop + interleaved finish ----------
            dmin2 = fin.tile([128, NPAIR * N], BF)
            dist = fin.tile([128, NPAIR * N], FD)
            part = fin.tile([128, 3], FD)
            zb128 = fin.tile([128, 1], FD)
            nc.vector.memset(zb128, 0.0)

            def finish_chunk(ci, p0, np_):
                c0 = p0 * N
                w = np_ * N
                dsl = bass.AP(tensor=dmin2.tensor, offset=dmin2.offset + c0,
                              ap=[dmin2.ap[0], [1, w]])
                nc.vector.tensor_scalar(out=dsl, in0=dsl, scalar1=0.0,
                                        scalar2=None, op0=AL.max)
                dstl = bass.AP(tensor=dist.tensor, offset=dist.offset + c0,
                               ap=[dist.ap[0], [1, w]])
                nc.scalar.activation(out=dstl, in_=dsl, func=AF.Sqrt,
                                     bias=zb128)
                t2 = mtmp.tile([128, w], FD, tag=f"t2{ci}")
                u = mtmp.tile([128, w], FD, tag=f"u{ci}")
                wi_ap = bass.AP(tensor=WI.tensor, offset=WI.offset + c0,
                                ap=[WI.ap[0], [1, w]])
                wm_ap = bass.AP(tensor=WM.tensor, offset=WM.offset + c0,
                                ap=[WM.ap[0], [1, w]])
                if ci <= 1:
                    nc.gpsimd.tensor_tensor(out=t2, in0=dstl, in1=wi_ap,
                                            op=AL.mult)
                    nc.gpsimd.tensor_tensor(out=u, in0=wm_ap, in1=t2,
                                            op=AL.subtract)
                else:
                    nc.vector.tensor_tensor(out=t2, in0=dstl, in1=wi_ap,
                                            op=AL.mult)
                    nc.vector.tensor_tensor(out=u, in0=wm_ap, in1=t2,
                                            op=AL.subtract)
                nc.scalar.activation(out=u, in_=u, func=AF.Relu, bias=zb128,
                                     accum_out=part[:, ci:ci + 1])

            for p in range(NPAIR):
                P3 = p3pool.tile([128, 3 * 512], FD, tag="P3")
                P2 = p2pool.tile([128, 2 * 512], FD, tag="P2")
                for l in range(5):
                    dst = P3 if l < 3 else P2
                    c0 = 512 * l if l < 3 else 512 * (l - 3)
                    nc.tensor.matmul(
                        out=dst[0:128, c0:c0 + AK],
                        lhsT=bass.AP(tensor=SL.tensor,
                                     offset=SL.offset + 640 * p + l,
                                     ap=[SL.ap[0], [AK, 2], [K, N]]),
                        rhs=V[0:2 * NR, AK * p:AK * (p + 1)],
                        tile_position=(0, 0))

                dslice = bass.AP(tensor=dmin2.tensor,
                                 offset=dmin2.offset + p * N,
                                 ap=[dmin2.ap[0], [1, N]])
                # flat bf16 D: cols l*320 + i*5 + k
                Dt = dtile.tile([128, 5 * AK], BF, tag="D")
                nc.scalar.activation(
                    out=bass.AP(tensor=Dt.tensor, offset=Dt.offset,
                                ap=[Dt.ap[0], [AK, 3], [1, AK]]),
                    in_=bass.AP(tensor=P3.tensor, offset=P3.offset,
                                ap=[P3.ap[0], [512, 3], [1, AK]]),
                    func=AF.Copy)
                nc.scalar.activation(
                    out=bass.AP(tensor=Dt.tensor, offset=Dt.offset + 3 * AK,
                                ap=[Dt.ap[0], [AK, 2], [1, AK]]),
                    in_=bass.AP(tensor=P2.tensor, offset=P2.offset,
                                ap=[P2.ap[0], [512, 2], [1, AK]]),
                    func=AF.Copy)
                nc.vector.tensor_reduce(
                    out=dslice,
                    in_=bass.AP(tensor=Dt.tensor, offset=Dt.offset,
                                ap=[Dt.ap[0], [K, N], [AK, 5], [1, K]]),
                    axis=mybir.AxisListType.XY, op=AL.min)

                if p == 9:
                    finish_chunk(0, 0, 10)
                elif p == 18:
                    finish_chunk(1, 10, 9)
                elif p == 19:
                    finish_chunk(2, 19, 1)

            nc.sync.dma_start(out=part_out, in_=part)

    nc.compile()
    return nc


def kernel(Y, length, width):
    Y = np.asarray(Y, np.float32)
    length = np.asarray(length, np.float32)
    width = np.asarray(width, np.float32)

    if "nc" not in _CACHE:
        _CACHE["nc"] = _build()
    nc = _CACHE["nc"]

    f2 = (2.0 * np.arange(K, dtype=np.float32) / (K - 1) - 1.0)
    ew = DECAY_RATE ** np.arange(T, dtype=np.float32)
    ew = (ew / ew.sum()).astype(np.float64)

    # SL constant image: zeros + ones rows (r2, r3 per h) in the h-col block
    ZI = np.zeros((2 * NR, SLW), ml_dtypes.bfloat16)
    for h in range(2):
        for r in (2, 3):
            row = np.zeros(SLW, np.float32).reshape(NPAIR, 2, AK)
            row[:, h, :] = 1.0
            ZI[h * NR + r] = row.reshape(SLW).astype(ml_dtypes.bfloat16)
    ONES = np.ones(NPAIR * AK, ml_dtypes.bfloat16)

    # prep-row rr = h*20 + p  <->  local slab t_local = 2p + h
    rr = np.arange(TL)
    tl_of_rr = 2 * (rr % NPAIR) + rr // NPAIR

    in_maps = []
    for c in range(NCORES):
        b, th = divmod(c, 2)
        t0 = th * TL
        tglob = t0 + tl_of_rr                       # [TL] global t per row

        yt = np.empty((TL, 3 * N), np.float32)
        yt[:, 0:N] = Y[b, :, tglob, 0]              # x[t, a]
        yt[:, N:2 * N] = Y[b, :, tglob, 1]          # y
        yt[:, 2 * N:3 * N] = Y[b, :, tglob, 4]      # yaw

        rad = width[b] / 2.0
        cmax = length[b] / 2.0 - rad                # [N]
        ck = (cmax[:, None] * f2[None, :]).reshape(AK).astype(np.float32)

        pd = rad[:, None] + rad[None, :] + BUFFER_DIST   # [j, i]
        ip = np.concatenate([1.0 / pd, 1.0 / pd], axis=0)  # [128, 64]

        wm = np.zeros((128, NPAIR * N), np.float64)
        mask = (~np.eye(N, dtype=bool)).astype(np.float64)   # [j, i]
        for p in range(NPAIR):
            for h in range(2):
                t = t0 + 2 * p + h
                wm[h * N:(h + 1) * N, p * N:(p + 1) * N] = \
                    mask * (ew[t] / (B * N * T))
        wi = wm * np.tile(ip, (1, NPAIR)).astype(np.float64)

        in_maps.append({
            "yt_in": yt, "ck_in": ck,
            "wm_in": wm.astype(np.float32),
            "wi_in": wi.astype(np.float32),
            "zi_in": ZI, "on_in": ONES,
        })

    global _LAST_INMAPS
    _LAST_INMAPS = in_maps
    res = bass_utils.run_bass_kernel_spmd(nc, in_maps,
                                          core_ids=list(range(NCORES)))
    total = 0.0
    for c in range(NCORES):
        total += float(res.results[c]["part_out"].astype(np.float64).sum())
    return np.float32(total)
